# revision 59
# baseline (speedup 1.0000x reference)
"""Trainium2 Bass kernel for nn_DigitCap (sparse_attention).

Math note: the reference's softmax is over a size-1 axis, so C == 1 exactly
and the whole N x N attention matrix A is dead code.  The computation
collapses to

    S[b,d,i]  = sum_{n,j} (1 + B[d,n]) * W[d,n,i,j] * U[b,n,j]
    out[b,d,:] = (1 - exp(-|S|)) * S / (|S| + 1e-7)

Sharding: split by digit capsule d (2 of 10 per core, zero-padded to a
uniform 2 so the SPMD program is identical on all 8 cores).  Each core then
reads only 262KB of W plus the replicated 1MB U^T -- 1.26MB/core instead of
the 2.77MB a batch shard would need, halving the HBM-bound streaming phase.

Written in raw Bass (explicit semaphores): the Tile framework's tail drain
emits more sem waits per instruction than this toolchain's codegen accepts.
"""

import numpy as np
from contextlib import ExitStack

import concourse.bass as bass
import concourse.mybir as mybir
from concourse.bass_utils import run_bass_kernel_spmd

F32 = mybir.dt.float32
AF = mybir.ActivationFunctionType
P = 128
D, DD, N, DP = 10, 16, 512, 8     # digit caps, digit dim, primary caps, primary dim
K = N * DP                         # 4096 contraction
NCHUNK = K // P                    # 32 chunks of 128 contraction rows
NCORES = 8
BFULL = 64
DC = 2                             # d's per core (8*2 = 16 slots >= 10 real)
DIC = DC * DD                      # 32 output cols per core
NUG = 8                            # U DMA groups
GC = NCHUNK // NUG                 # 4 chunks per U group
EPS = 1e-7


def build_raw():
    nc = bass.Bass()
    u_t = nc.dram_tensor("u_t", [P, NCHUNK * BFULL], F32, kind="ExternalInput")
    w_t = nc.dram_tensor("w_t", [P, NCHUNK * DIC], F32, kind="ExternalInput")
    bp = nc.dram_tensor("bp", [P, NCHUNK * DC], F32, kind="ExternalInput")
    out = nc.dram_tensor("out", [BFULL, DIC], F32, kind="ExternalOutput")

    with ExitStack() as ctx:
        u_all = ctx.enter_context(nc.sbuf_tensor("u_all", [P, NCHUNK * BFULL], F32))
        w_all = ctx.enter_context(nc.sbuf_tensor("w_all", [P, NCHUNK * DIC], F32))
        bsc = ctx.enter_context(nc.sbuf_tensor("bsc", [P, NCHUNK * DC], F32))
        ps = ctx.enter_context(nc.psum_tensor("ps", [BFULL, DIC], F32))
        psb = ctx.enter_context(nc.psum_tensor("psb", [BFULL, DIC], F32))
        s = ctx.enter_context(nc.sbuf_tensor("s", [BFULL, DIC], F32))
        sq = ctx.enter_context(nc.sbuf_tensor("sq", [BFULL, DIC], F32))
        ss = ctx.enter_context(nc.sbuf_tensor("ss", [BFULL, DC], F32))
        normt = ctx.enter_context(nc.sbuf_tensor("norm", [BFULL, DC], F32))
        den = ctx.enter_context(nc.sbuf_tensor("den", [BFULL, DC], F32))
        rec = ctx.enter_context(nc.sbuf_tensor("rec", [BFULL, DC], F32))
        et = ctx.enter_context(nc.sbuf_tensor("et", [BFULL, DC], F32))
        numt = ctx.enter_context(nc.sbuf_tensor("numt", [BFULL, DC], F32))
        ot = ctx.enter_context(nc.sbuf_tensor("ot", [BFULL, DIC], F32))
        warm = ctx.enter_context(nc.sbuf_tensor("warm", [1, 4], F32))
        sem_w = [ctx.enter_context(nc.semaphore(f"sem_w{h}")) for h in range(2)]
        sem_bc = ctx.enter_context(nc.semaphore("sem_bc"))
        sem_ug = [ctx.enter_context(nc.semaphore(f"sem_ug{g}")) for g in range(NUG)]
        sem_dve = ctx.enter_context(nc.semaphore("sem_dve"))
        sem_pe = ctx.enter_context(nc.semaphore("sem_pe"))
        sem_pe2 = ctx.enter_context(nc.semaphore("sem_pe2"))
        sem_v2 = ctx.enter_context(nc.semaphore("sem_v2"))
        sem_act1 = ctx.enter_context(nc.semaphore("sem_act1"))
        sem_act2 = ctx.enter_context(nc.semaphore("sem_act2"))
        sem_fin = ctx.enter_context(nc.semaphore("sem_fin"))
        sem_out = ctx.enter_context(nc.semaphore("sem_out"))
        sem_wm = ctx.enter_context(nc.semaphore("sem_wm"))
        sem_s1 = ctx.enter_context(nc.semaphore("sem_s1"))
        sem_c2 = ctx.enter_context(nc.semaphore("sem_c2"))
        sem_c4 = ctx.enter_context(nc.semaphore("sem_c4"))

        with nc.Block() as block:

            @block.sync
            def _(sync):
                # W halves first: they gate the scale -> PE start
                HC = NCHUNK // 2
                for h in range(2):
                    sync.dma_start(
                        w_all[:, h * HC * DIC:(h + 1) * HC * DIC],
                        bass.AP(
                            w_t, h * HC * DIC,
                            [[NCHUNK * DIC, P], [1, HC * DIC]],
                        ),
                    ).then_inc(sem_w[h], 16)
                # U^T streamed in 8 groups of 4 chunks: contiguous 16KB runs
                for g in range(NUG):
                    sync.dma_start(
                        u_all[:, g * GC * BFULL:(g + 1) * GC * BFULL],
                        bass.AP(
                            u_t, g * GC * BFULL,
                            [[NCHUNK * BFULL, P], [1, GC * BFULL]],
                        ),
                    ).then_inc(sem_ug[g], 16)
                # output
                sync.wait_ge(sem_fin, 1)
                sync.dma_start(out[:, :], ot[:]).then_inc(sem_out, 16)
                sync.wait_ge(sem_out, 16)

            @block.vector
            def _(vector):
                # seed for the ACT table warm-up
                vector.memset(warm[:], 1.0).then_inc(sem_wm, 1)
                # fused (bsc + 1) * W in two halves so PE can start early
                vector.wait_ge(sem_bc, 16)
                HC = NCHUNK // 2
                for h in range(2):
                    vector.wait_ge(sem_w[h], 16)
                    w_v = w_all[:, h * HC * DIC:(h + 1) * HC * DIC].rearrange(
                        "p (c t i) -> p c t i", t=DC, i=DD
                    )
                    vector.scalar_tensor_tensor(
                        out=w_v,
                        in0=bsc[:, h * HC * DC:(h + 1) * HC * DC]
                        .rearrange("p (c t) -> p c t", t=DC)
                        .broadcast_to([P, HC, DC, DD]),
                        scalar=1.0,
                        in1=w_v,
                        op0=mybir.AluOpType.add,
                        op1=mybir.AluOpType.mult,
                    ).then_inc(sem_dve, 1)
                # epilogue part 1: s = ps(copied by ACT) + psb, squares, sums
                vector.wait_ge(sem_s1, 1)
                vector.wait_ge(sem_pe, 1)
                vector.tensor_add(out=s[:], in0=s[:], in1=psb[:]).then_inc(
                    sem_c2, 1
                )
                vector.wait_ge(sem_c2, 1)
                s3 = s[:].rearrange("b (t i) -> b t i", i=DD)
                vector.tensor_mul(
                    out=sq[:].rearrange("b (t i) -> b t i", i=DD), in0=s3, in1=s3
                ).then_inc(sem_c2, 1)
                vector.wait_ge(sem_c2, 2)
                vector.tensor_reduce(
                    out=ss[:], in_=sq[:].rearrange("b (t i) -> b t i", i=DD),
                    axis=mybir.AxisListType.X, op=mybir.AluOpType.add,
                ).then_inc(sem_v2, 1)
                # den/rec/o1 under the Exp table load
                vector.wait_ge(sem_act1, 1)
                vector.tensor_scalar_add(
                    out=den[:], in0=normt[:], scalar1=EPS
                ).then_inc(sem_c4, 1)
                vector.wait_ge(sem_c4, 1)
                vector.reciprocal(out=rec[:], in_=den[:]).then_inc(sem_c4, 1)
                vector.wait_ge(sem_c4, 2)
                vector.tensor_mul(
                    out=ot[:].rearrange("b (t i) -> b t i", i=DD),
                    in0=s3, in1=rec[:].broadcast_to([BFULL, DC, DD]),
                ).then_inc(sem_c4, 1)
                vector.wait_ge(sem_act2, 1)
                vector.tensor_scalar(
                    out=numt[:], in0=et[:], scalar1=-1.0, scalar2=1.0,
                    op0=mybir.AluOpType.mult, op1=mybir.AluOpType.add,
                ).then_inc(sem_c4, 1)
                vector.wait_ge(sem_c4, 4)
                o3 = ot[:].rearrange("b (t i) -> b t i", i=DD)
                vector.tensor_mul(
                    out=o3, in0=o3, in1=numt[:].broadcast_to([BFULL, DC, DD]),
                ).then_inc(sem_fin, 1)

            @block.tensor
            def _(tensor):
                for g in range(NUG):
                    if g == 0:
                        tensor.wait_ge(sem_dve, 1)
                    elif g == NUG // 2:
                        tensor.wait_ge(sem_dve, 2)
                    tensor.wait_ge(sem_ug[g], 16)
                    for k in range(GC):
                        c = g * GC + k
                        # alternate PSUM banks so consecutive matmuls pipeline
                        tgt = ps if c % 2 == 0 else psb
                        mm = tensor.matmul(
                            tgt[:],
                            lhsT=u_all[:, c * BFULL:(c + 1) * BFULL],
                            rhs=w_all[:, c * DIC:(c + 1) * DIC],
                            start=(c < 2), stop=(c >= NCHUNK - 2),
                            skip_group_check=True,
                        )
                # explicit drain: guarantees both PSUM banks are complete,
                # and fires the epilogue sem without waiting for the
                # Block-exit dge-drain
                tensor.drain().then_inc(sem_pe, 1)

            @block.scalar
            def _(scalar):
                # bsc on the ACT HWDGE ring (W + U own the SP ring)
                scalar.dma_start(bsc[:], bp[:, :]).then_inc(sem_bc, 16)
                # ACT table warm-up (Copy shares the Sqrt table)
                scalar.wait_ge(sem_wm, 1)
                scalar.activation(out=warm[:, 0:1], in_=warm[:, 1:2], func=AF.Sqrt)
                # epilogue: S copy, norm, exp(-norm)
                scalar.wait_ge(sem_pe, 1)
                scalar.activation(out=s[:], in_=ps[:], func=AF.Copy).then_inc(
                    sem_s1, 1
                )
                scalar.wait_ge(sem_v2, 1)
                scalar.activation(out=normt[:], in_=ss[:], func=AF.Sqrt).then_inc(
                    sem_act1, 1
                )
                scalar.wait_ge(sem_act1, 1)
                scalar.activation(
                    out=et[:], in_=normt[:], func=AF.Exp, scale=-1.0
                ).then_inc(sem_act2, 1)

    return nc


_CACHE = {}


def _get_nc():
    if "nc" not in _CACHE:
        _CACHE["nc"] = build_raw()
    return _CACHE["nc"]


def prep_inputs(primary_caps, W, B):
    """Host-side layout prep + sharding (no arithmetic).

    Contraction row order: chunk c holds n in [c*16, (c+1)*16); within a
    chunk, partition p = j*16 + n_local.  Core c owns digit caps
    d in {2c, 2c+1} (zeros for the 6 pad slots on cores 5-7).
    """
    U = np.asarray(primary_caps, dtype=np.float32)
    Wf = np.asarray(W, dtype=np.float32)
    Bf = np.asarray(B, dtype=np.float32).reshape(D, N)

    # U^T replicated: [p, (c b)]
    Unj = np.transpose(U, (1, 2, 0))  # n j b
    Ut = np.ascontiguousarray(
        Unj.reshape(NCHUNK, 16, DP, BFULL)
        .transpose(0, 2, 1, 3)
        .reshape(NCHUNK, P, BFULL)
        .transpose(1, 0, 2)
        .reshape(P, NCHUNK * BFULL)
    )

    # per-core W slice [p, (c, t, i)] and B slice [p, (c, t)]
    Wnj = np.transpose(Wf, (1, 3, 0, 2))  # n j d i
    Wc = (
        Wnj.reshape(NCHUNK, 16, DP, D, DD)
        .transpose(0, 2, 1, 3, 4)          # c j n_l d i
        .reshape(NCHUNK, P, D, DD)
        .transpose(1, 0, 2, 3)             # p c d i
    )
    Bn = Bf.reshape(D, NCHUNK, 16)         # d c n_l

    in_maps = []
    for core in range(NCORES):
        wt = np.zeros((P, NCHUNK, DC, DD), dtype=np.float32)
        bpt = np.zeros((16, NCHUNK, DC), dtype=np.float32)
        for t in range(DC):
            d = 2 * core + t
            if d < D:
                wt[:, :, t, :] = Wc[:, :, d, :]
                bpt[:, :, t] = Bn[d].T      # [n_l, c] -> ...
        bpm = np.ascontiguousarray(
            np.broadcast_to(
                bpt.reshape(1, 16, NCHUNK * DC), (DP, 16, NCHUNK * DC)
            ).reshape(P, NCHUNK * DC)
        )
        in_maps.append(
            {
                "u_t": Ut,
                "w_t": np.ascontiguousarray(wt.reshape(P, NCHUNK * DIC)),
                "bp": bpm,
            }
        )
    return in_maps


def kernel(primary_caps, W, B):
    nc = _get_nc()
    in_maps = prep_inputs(primary_caps, W, B)
    res = run_bass_kernel_spmd(nc, in_maps, core_ids=list(range(NCORES)))
    full = np.empty((BFULL, D, DD), dtype=np.float32)
    for core in range(NCORES):
        o = res.results[core]["out"].reshape(BFULL, DC, DD)
        for t in range(DC):
            d = 2 * core + t
            if d < D:
                full[:, d, :] = o[:, t, :]
    return full


# revision 60
# speedup vs baseline: 1.0145x; 1.0145x over previous
"""Trainium2 Bass kernel for nn_DigitCap (sparse_attention).

Math note: the reference's softmax is over a size-1 axis, so C == 1 exactly
and the whole N x N attention matrix A is dead code.  The computation
collapses to

    S[b,d,i]  = sum_{n,j} (1 + B[d,n]) * W[d,n,i,j] * U[b,n,j]
    out[b,d,:] = (1 - exp(-|S|)) * S / (|S| + 1e-7)

Sharding: split by digit capsule d (2 of 10 per core, zero-padded to a
uniform 2 so the SPMD program is identical on all 8 cores).  Each core then
reads only 262KB of W plus the replicated 1MB U^T -- 1.26MB/core instead of
the 2.77MB a batch shard would need, halving the HBM-bound streaming phase.

Written in raw Bass (explicit semaphores): the Tile framework's tail drain
emits more sem waits per instruction than this toolchain's codegen accepts.
"""

import numpy as np
from contextlib import ExitStack

import concourse.bass as bass
import concourse.mybir as mybir
from concourse.bass_utils import run_bass_kernel_spmd

F32 = mybir.dt.float32
AF = mybir.ActivationFunctionType
P = 128
D, DD, N, DP = 10, 16, 512, 8     # digit caps, digit dim, primary caps, primary dim
K = N * DP                         # 4096 contraction
NCHUNK = K // P                    # 32 chunks of 128 contraction rows
NCORES = 8
BFULL = 64
DC = 2                             # d's per core (8*2 = 16 slots >= 10 real)
DIC = DC * DD                      # 32 output cols per core
NUG = 8                            # U DMA groups
GC = NCHUNK // NUG                 # 4 chunks per U group
EPS = 1e-7


def build_raw():
    nc = bass.Bass()
    u_t = nc.dram_tensor("u_t", [P, NCHUNK * BFULL], F32, kind="ExternalInput")
    w_t = nc.dram_tensor("w_t", [P, NCHUNK * DIC], F32, kind="ExternalInput")
    bp = nc.dram_tensor("bp", [P, NCHUNK * DC], F32, kind="ExternalInput")
    out = nc.dram_tensor("out", [BFULL, DIC], F32, kind="ExternalOutput")

    with ExitStack() as ctx:
        u_all = ctx.enter_context(nc.sbuf_tensor("u_all", [P, NCHUNK * BFULL], F32))
        w_all = ctx.enter_context(nc.sbuf_tensor("w_all", [P, NCHUNK * DIC], F32))
        bsc = ctx.enter_context(nc.sbuf_tensor("bsc", [P, NCHUNK * DC], F32))
        ps = ctx.enter_context(nc.psum_tensor("ps", [BFULL, DIC], F32))
        psb = ctx.enter_context(nc.psum_tensor("psb", [BFULL, DIC], F32))
        s = ctx.enter_context(nc.sbuf_tensor("s", [BFULL, DIC], F32))
        sq = ctx.enter_context(nc.sbuf_tensor("sq", [BFULL, DIC], F32))
        ss = ctx.enter_context(nc.sbuf_tensor("ss", [BFULL, DC], F32))
        normt = ctx.enter_context(nc.sbuf_tensor("norm", [BFULL, DC], F32))
        den = ctx.enter_context(nc.sbuf_tensor("den", [BFULL, DC], F32))
        rec = ctx.enter_context(nc.sbuf_tensor("rec", [BFULL, DC], F32))
        et = ctx.enter_context(nc.sbuf_tensor("et", [BFULL, DC], F32))
        numt = ctx.enter_context(nc.sbuf_tensor("numt", [BFULL, DC], F32))
        ot = ctx.enter_context(nc.sbuf_tensor("ot", [BFULL, DIC], F32))
        warm = ctx.enter_context(nc.sbuf_tensor("warm", [1, 4], F32))
        sem_w = [ctx.enter_context(nc.semaphore(f"sem_w{h}")) for h in range(2)]
        sem_bc = ctx.enter_context(nc.semaphore("sem_bc"))
        sem_ug = [ctx.enter_context(nc.semaphore(f"sem_ug{g}")) for g in range(NUG)]
        sem_dve = ctx.enter_context(nc.semaphore("sem_dve"))
        sem_pe = ctx.enter_context(nc.semaphore("sem_pe"))
        sem_pe2 = ctx.enter_context(nc.semaphore("sem_pe2"))
        sem_v2 = ctx.enter_context(nc.semaphore("sem_v2"))
        sem_act1 = ctx.enter_context(nc.semaphore("sem_act1"))
        sem_act2 = ctx.enter_context(nc.semaphore("sem_act2"))
        sem_fin = ctx.enter_context(nc.semaphore("sem_fin"))
        sem_out = ctx.enter_context(nc.semaphore("sem_out"))
        sem_wm = ctx.enter_context(nc.semaphore("sem_wm"))
        sem_s1 = ctx.enter_context(nc.semaphore("sem_s1"))
        sem_c2 = ctx.enter_context(nc.semaphore("sem_c2"))
        sem_c4 = ctx.enter_context(nc.semaphore("sem_c4"))

        with nc.Block() as block:

            @block.sync
            def _(sync):
                # W halves first: they gate the scale -> PE start
                HC = NCHUNK // 2
                for h in range(2):
                    sync.dma_start(
                        w_all[:, h * HC * DIC:(h + 1) * HC * DIC],
                        bass.AP(
                            w_t, h * HC * DIC,
                            [[NCHUNK * DIC, P], [1, HC * DIC]],
                        ),
                    ).then_inc(sem_w[h], 16)
                # U^T streamed in 8 groups of 4 chunks: contiguous 16KB runs
                for g in range(NUG):
                    sync.dma_start(
                        u_all[:, g * GC * BFULL:(g + 1) * GC * BFULL],
                        bass.AP(
                            u_t, g * GC * BFULL,
                            [[NCHUNK * BFULL, P], [1, GC * BFULL]],
                        ),
                    ).then_inc(sem_ug[g], 16)
                # output
                sync.wait_ge(sem_fin, 1)
                sync.dma_start(out[:, :], ot[:]).then_inc(sem_out, 16)
                sync.wait_ge(sem_out, 16)

            @block.vector
            def _(vector):
                # seed for the ACT table warm-up
                vector.memset(warm[:], 1.0).then_inc(sem_wm, 1)
                # fused (bsc + 1) * W in two halves so PE can start early
                vector.wait_ge(sem_bc, 16)
                HC = NCHUNK // 2
                for h in range(2):
                    vector.wait_ge(sem_w[h], 16)
                    w_v = w_all[:, h * HC * DIC:(h + 1) * HC * DIC].rearrange(
                        "p (c t i) -> p c t i", t=DC, i=DD
                    )
                    vector.scalar_tensor_tensor(
                        out=w_v,
                        in0=bsc[:, h * HC * DC:(h + 1) * HC * DC]
                        .rearrange("p (c t) -> p c t", t=DC)
                        .broadcast_to([P, HC, DC, DD]),
                        scalar=1.0,
                        in1=w_v,
                        op0=mybir.AluOpType.add,
                        op1=mybir.AluOpType.mult,
                    ).then_inc(sem_dve, 1)
                # epilogue part 1: s = ps(copied by ACT) + psb, squares, sums
                vector.wait_ge(sem_s1, 1)
                vector.wait_ge(sem_pe2, 1)
                vector.tensor_add(out=s[:], in0=s[:], in1=psb[:]).then_inc(
                    sem_c2, 1
                )
                vector.wait_ge(sem_c2, 1)
                s3 = s[:].rearrange("b (t i) -> b t i", i=DD)
                vector.tensor_mul(
                    out=sq[:].rearrange("b (t i) -> b t i", i=DD), in0=s3, in1=s3
                ).then_inc(sem_c2, 1)
                vector.wait_ge(sem_c2, 2)
                vector.tensor_reduce(
                    out=ss[:], in_=sq[:].rearrange("b (t i) -> b t i", i=DD),
                    axis=mybir.AxisListType.X, op=mybir.AluOpType.add,
                ).then_inc(sem_v2, 1)
                # den/rec/o1 under the Exp table load
                vector.wait_ge(sem_act1, 1)
                vector.tensor_scalar_add(
                    out=den[:], in0=normt[:], scalar1=EPS
                ).then_inc(sem_c4, 1)
                vector.wait_ge(sem_c4, 1)
                vector.reciprocal(out=rec[:], in_=den[:]).then_inc(sem_c4, 1)
                vector.wait_ge(sem_c4, 2)
                vector.tensor_mul(
                    out=ot[:].rearrange("b (t i) -> b t i", i=DD),
                    in0=s3, in1=rec[:].broadcast_to([BFULL, DC, DD]),
                ).then_inc(sem_c4, 1)
                vector.wait_ge(sem_act2, 1)
                vector.tensor_scalar(
                    out=numt[:], in0=et[:], scalar1=-1.0, scalar2=1.0,
                    op0=mybir.AluOpType.mult, op1=mybir.AluOpType.add,
                ).then_inc(sem_c4, 1)
                vector.wait_ge(sem_c4, 4)
                o3 = ot[:].rearrange("b (t i) -> b t i", i=DD)
                vector.tensor_mul(
                    out=o3, in0=o3, in1=numt[:].broadcast_to([BFULL, DC, DD]),
                ).then_inc(sem_fin, 1)

            @block.tensor
            def _(tensor):
                for g in range(NUG):
                    if g == 0:
                        tensor.wait_ge(sem_dve, 1)
                    elif g == NUG // 2:
                        tensor.wait_ge(sem_dve, 2)
                    tensor.wait_ge(sem_ug[g], 16)
                    for k in range(GC):
                        c = g * GC + k
                        # alternate PSUM banks so consecutive matmuls pipeline
                        tgt = ps if c % 2 == 0 else psb
                        mm = tensor.matmul(
                            tgt[:],
                            lhsT=u_all[:, c * BFULL:(c + 1) * BFULL],
                            rhs=w_all[:, c * DIC:(c + 1) * DIC],
                            start=(c < 2), stop=(c >= NCHUNK - 2),
                            skip_group_check=True,
                        )
                        if c == NCHUNK - 2:
                            # last write to ps: unblocks the ACT copy without
                            # waiting for the final matmul + engine drain
                            mm.then_inc(sem_pe, 1)
                mm.then_inc(sem_pe2, 1)

            @block.scalar
            def _(scalar):
                # bsc on the ACT HWDGE ring (W + U own the SP ring)
                scalar.dma_start(bsc[:], bp[:, :]).then_inc(sem_bc, 16)
                # ACT table warm-up (Copy shares the Sqrt table)
                scalar.wait_ge(sem_wm, 1)
                scalar.activation(out=warm[:, 0:1], in_=warm[:, 1:2], func=AF.Sqrt)
                # epilogue: S copy, norm, exp(-norm)
                scalar.wait_ge(sem_pe, 1)
                scalar.activation(out=s[:], in_=ps[:], func=AF.Copy).then_inc(
                    sem_s1, 1
                )
                scalar.wait_ge(sem_v2, 1)
                scalar.activation(out=normt[:], in_=ss[:], func=AF.Sqrt).then_inc(
                    sem_act1, 1
                )
                scalar.wait_ge(sem_act1, 1)
                scalar.activation(
                    out=et[:], in_=normt[:], func=AF.Exp, scale=-1.0
                ).then_inc(sem_act2, 1)

    return nc


_CACHE = {}


def _get_nc():
    if "nc" not in _CACHE:
        _CACHE["nc"] = build_raw()
    return _CACHE["nc"]


def prep_inputs(primary_caps, W, B):
    """Host-side layout prep + sharding (no arithmetic).

    Contraction row order: chunk c holds n in [c*16, (c+1)*16); within a
    chunk, partition p = j*16 + n_local.  Core c owns digit caps
    d in {2c, 2c+1} (zeros for the 6 pad slots on cores 5-7).
    """
    U = np.asarray(primary_caps, dtype=np.float32)
    Wf = np.asarray(W, dtype=np.float32)
    Bf = np.asarray(B, dtype=np.float32).reshape(D, N)

    # U^T replicated: [p, (c b)]
    Unj = np.transpose(U, (1, 2, 0))  # n j b
    Ut = np.ascontiguousarray(
        Unj.reshape(NCHUNK, 16, DP, BFULL)
        .transpose(0, 2, 1, 3)
        .reshape(NCHUNK, P, BFULL)
        .transpose(1, 0, 2)
        .reshape(P, NCHUNK * BFULL)
    )

    # per-core W slice [p, (c, t, i)] and B slice [p, (c, t)]
    Wnj = np.transpose(Wf, (1, 3, 0, 2))  # n j d i
    Wc = (
        Wnj.reshape(NCHUNK, 16, DP, D, DD)
        .transpose(0, 2, 1, 3, 4)          # c j n_l d i
        .reshape(NCHUNK, P, D, DD)
        .transpose(1, 0, 2, 3)             # p c d i
    )
    Bn = Bf.reshape(D, NCHUNK, 16)         # d c n_l

    in_maps = []
    for core in range(NCORES):
        wt = np.zeros((P, NCHUNK, DC, DD), dtype=np.float32)
        bpt = np.zeros((16, NCHUNK, DC), dtype=np.float32)
        for t in range(DC):
            d = 2 * core + t
            if d < D:
                wt[:, :, t, :] = Wc[:, :, d, :]
                bpt[:, :, t] = Bn[d].T      # [n_l, c] -> ...
        bpm = np.ascontiguousarray(
            np.broadcast_to(
                bpt.reshape(1, 16, NCHUNK * DC), (DP, 16, NCHUNK * DC)
            ).reshape(P, NCHUNK * DC)
        )
        in_maps.append(
            {
                "u_t": Ut,
                "w_t": np.ascontiguousarray(wt.reshape(P, NCHUNK * DIC)),
                "bp": bpm,
            }
        )
    return in_maps


def kernel(primary_caps, W, B):
    nc = _get_nc()
    in_maps = prep_inputs(primary_caps, W, B)
    res = run_bass_kernel_spmd(nc, in_maps, core_ids=list(range(NCORES)))
    full = np.empty((BFULL, D, DD), dtype=np.float32)
    for core in range(NCORES):
        o = res.results[core]["out"].reshape(BFULL, DC, DD)
        for t in range(DC):
            d = 2 * core + t
            if d < D:
                full[:, d, :] = o[:, t, :]
    return full


# revision 61
# speedup vs baseline: 1.0672x; 1.0520x over previous
"""Trainium2 Bass kernel for nn_DigitCap (sparse_attention).

Math note: the reference's softmax is over a size-1 axis, so C == 1 exactly
and the whole N x N attention matrix A is dead code.  The computation
collapses to

    S[b,d,i]  = sum_{n,j} (1 + B[d,n]) * W[d,n,i,j] * U[b,n,j]
    out[b,d,:] = (1 - exp(-|S|)) * S / (|S| + 1e-7)

Sharding: split by digit capsule d (2 of 10 per core, zero-padded to a
uniform 2 so the SPMD program is identical on all 8 cores).  Each core then
reads only 262KB of W plus the replicated 1MB U^T -- 1.26MB/core instead of
the 2.77MB a batch shard would need, halving the HBM-bound streaming phase.

Written in raw Bass (explicit semaphores): the Tile framework's tail drain
emits more sem waits per instruction than this toolchain's codegen accepts.
"""

import numpy as np
from contextlib import ExitStack

import concourse.bass as bass
import concourse.mybir as mybir
from concourse.bass_utils import run_bass_kernel_spmd

F32 = mybir.dt.float32
AF = mybir.ActivationFunctionType
P = 128
D, DD, N, DP = 10, 16, 512, 8     # digit caps, digit dim, primary caps, primary dim
K = N * DP                         # 4096 contraction
NCHUNK = K // P                    # 32 chunks of 128 contraction rows
NCORES = 8
BFULL = 64
DC = 2                             # d's per core (8*2 = 16 slots >= 10 real)
DIC = DC * DD                      # 32 output cols per core
NUG = 8                            # U DMA groups
GC = NCHUNK // NUG                 # 4 chunks per U group
EPS = 1e-7


def build_raw():
    nc = bass.Bass()
    u_t = nc.dram_tensor("u_t", [P, NCHUNK * BFULL], F32, kind="ExternalInput")
    w_t = nc.dram_tensor("w_t", [P, NCHUNK * DIC], F32, kind="ExternalInput")
    bp = nc.dram_tensor("bp", [P, NCHUNK * DC], F32, kind="ExternalInput")
    out = nc.dram_tensor("out", [BFULL, DIC], F32, kind="ExternalOutput")

    with ExitStack() as ctx:
        u_all = ctx.enter_context(nc.sbuf_tensor("u_all", [P, NCHUNK * BFULL], F32))
        w_all = ctx.enter_context(nc.sbuf_tensor("w_all", [P, NCHUNK * DIC], F32))
        bsc = ctx.enter_context(nc.sbuf_tensor("bsc", [P, NCHUNK * DC], F32))
        ps = ctx.enter_context(nc.psum_tensor("ps", [BFULL, DIC], F32))
        psb = ctx.enter_context(nc.psum_tensor("psb", [BFULL, DIC], F32))
        s = ctx.enter_context(nc.sbuf_tensor("s", [BFULL, DIC], F32))
        sq = ctx.enter_context(nc.sbuf_tensor("sq", [BFULL, DIC], F32))
        ss = ctx.enter_context(nc.sbuf_tensor("ss", [BFULL, DC], F32))
        normt = ctx.enter_context(nc.sbuf_tensor("norm", [BFULL, DC], F32))
        den = ctx.enter_context(nc.sbuf_tensor("den", [BFULL, DC], F32))
        rec = ctx.enter_context(nc.sbuf_tensor("rec", [BFULL, DC], F32))
        et = ctx.enter_context(nc.sbuf_tensor("et", [BFULL, DC], F32))
        numt = ctx.enter_context(nc.sbuf_tensor("numt", [BFULL, DC], F32))
        ot = ctx.enter_context(nc.sbuf_tensor("ot", [BFULL, DIC], F32))
        warm = ctx.enter_context(nc.sbuf_tensor("warm", [1, 4], F32))
        sem_w = [ctx.enter_context(nc.semaphore(f"sem_w{h}")) for h in range(2)]
        sem_bc = ctx.enter_context(nc.semaphore("sem_bc"))
        sem_ug = [ctx.enter_context(nc.semaphore(f"sem_ug{g}")) for g in range(NUG)]
        sem_dve = ctx.enter_context(nc.semaphore("sem_dve"))
        sem_pe = ctx.enter_context(nc.semaphore("sem_pe"))
        sem_pe2 = ctx.enter_context(nc.semaphore("sem_pe2"))
        sem_v2 = ctx.enter_context(nc.semaphore("sem_v2"))
        sem_act1 = ctx.enter_context(nc.semaphore("sem_act1"))
        sem_act2 = ctx.enter_context(nc.semaphore("sem_act2"))
        sem_fin = ctx.enter_context(nc.semaphore("sem_fin"))
        sem_out = ctx.enter_context(nc.semaphore("sem_out"))
        sem_wm = ctx.enter_context(nc.semaphore("sem_wm"))
        sem_s1 = ctx.enter_context(nc.semaphore("sem_s1"))
        sem_c2 = ctx.enter_context(nc.semaphore("sem_c2"))
        sem_c4 = ctx.enter_context(nc.semaphore("sem_c4"))

        with nc.Block() as block:

            @block.sync
            def _(sync):
                # W halves first: they gate the scale -> PE start
                HC = NCHUNK // 2
                for h in range(2):
                    sync.dma_start(
                        w_all[:, h * HC * DIC:(h + 1) * HC * DIC],
                        bass.AP(
                            w_t, h * HC * DIC,
                            [[NCHUNK * DIC, P], [1, HC * DIC]],
                        ),
                    ).then_inc(sem_w[h], 16)
                # U^T streamed in 8 groups of 4 chunks: contiguous 16KB runs
                for g in range(NUG):
                    sync.dma_start(
                        u_all[:, g * GC * BFULL:(g + 1) * GC * BFULL],
                        bass.AP(
                            u_t, g * GC * BFULL,
                            [[NCHUNK * BFULL, P], [1, GC * BFULL]],
                        ),
                    ).then_inc(sem_ug[g], 16)
                # output; completion is covered by the SP engine's exit
                # drain (same as Tile kernels), no explicit wait needed
                sync.wait_ge(sem_fin, 1)
                sync.dma_start(out[:, :], ot[:]).then_inc(sem_out, 16)

            @block.vector
            def _(vector):
                # seed for the ACT table warm-up
                vector.memset(warm[:], 1.0).then_inc(sem_wm, 1)
                # fused (bsc + 1) * W in two halves so PE can start early
                vector.wait_ge(sem_bc, 16)
                HC = NCHUNK // 2
                for h in range(2):
                    vector.wait_ge(sem_w[h], 16)
                    w_v = w_all[:, h * HC * DIC:(h + 1) * HC * DIC].rearrange(
                        "p (c t i) -> p c t i", t=DC, i=DD
                    )
                    vector.scalar_tensor_tensor(
                        out=w_v,
                        in0=bsc[:, h * HC * DC:(h + 1) * HC * DC]
                        .rearrange("p (c t) -> p c t", t=DC)
                        .broadcast_to([P, HC, DC, DD]),
                        scalar=1.0,
                        in1=w_v,
                        op0=mybir.AluOpType.add,
                        op1=mybir.AluOpType.mult,
                    ).then_inc(sem_dve, 1)
                # epilogue part 1: s = ps(copied by ACT) + psb, squares, sums
                vector.wait_ge(sem_s1, 1)
                vector.wait_ge(sem_pe2, 1)
                vector.tensor_add(out=s[:], in0=s[:], in1=psb[:]).then_inc(
                    sem_c2, 1
                )
                vector.wait_ge(sem_c2, 1)
                s3 = s[:].rearrange("b (t i) -> b t i", i=DD)
                vector.tensor_mul(
                    out=sq[:].rearrange("b (t i) -> b t i", i=DD), in0=s3, in1=s3
                ).then_inc(sem_c2, 1)
                vector.wait_ge(sem_c2, 2)
                vector.tensor_reduce(
                    out=ss[:], in_=sq[:].rearrange("b (t i) -> b t i", i=DD),
                    axis=mybir.AxisListType.X, op=mybir.AluOpType.add,
                ).then_inc(sem_v2, 1)
                # den/rec/o1 under the Exp table load
                vector.wait_ge(sem_act1, 1)
                vector.tensor_scalar_add(
                    out=den[:], in0=normt[:], scalar1=EPS
                ).then_inc(sem_c4, 1)
                vector.wait_ge(sem_c4, 1)
                vector.reciprocal(out=rec[:], in_=den[:]).then_inc(sem_c4, 1)
                vector.wait_ge(sem_c4, 2)
                vector.tensor_mul(
                    out=ot[:].rearrange("b (t i) -> b t i", i=DD),
                    in0=s3, in1=rec[:].broadcast_to([BFULL, DC, DD]),
                ).then_inc(sem_c4, 1)
                vector.wait_ge(sem_act2, 1)
                vector.tensor_scalar(
                    out=numt[:], in0=et[:], scalar1=-1.0, scalar2=1.0,
                    op0=mybir.AluOpType.mult, op1=mybir.AluOpType.add,
                ).then_inc(sem_c4, 1)
                vector.wait_ge(sem_c4, 4)
                o3 = ot[:].rearrange("b (t i) -> b t i", i=DD)
                vector.tensor_mul(
                    out=o3, in0=o3, in1=numt[:].broadcast_to([BFULL, DC, DD]),
                ).then_inc(sem_fin, 1)

            @block.tensor
            def _(tensor):
                for g in range(NUG):
                    if g == 0:
                        tensor.wait_ge(sem_dve, 1)
                    elif g == NUG // 2:
                        tensor.wait_ge(sem_dve, 2)
                    tensor.wait_ge(sem_ug[g], 16)
                    for k in range(GC):
                        c = g * GC + k
                        # alternate PSUM banks so consecutive matmuls pipeline
                        tgt = ps if c % 2 == 0 else psb
                        mm = tensor.matmul(
                            tgt[:],
                            lhsT=u_all[:, c * BFULL:(c + 1) * BFULL],
                            rhs=w_all[:, c * DIC:(c + 1) * DIC],
                            start=(c < 2), stop=(c >= NCHUNK - 2),
                            skip_group_check=True,
                        )
                        if c == NCHUNK - 2:
                            # last write to ps: unblocks the ACT copy without
                            # waiting for the final matmul + engine drain
                            mm.then_inc(sem_pe, 1)
                mm.then_inc(sem_pe2, 1)

            @block.scalar
            def _(scalar):
                # bsc on the ACT HWDGE ring (W + U own the SP ring)
                scalar.dma_start(bsc[:], bp[:, :]).then_inc(sem_bc, 16)
                # ACT table warm-up (Copy shares the Sqrt table)
                scalar.wait_ge(sem_wm, 1)
                scalar.activation(out=warm[:, 0:1], in_=warm[:, 1:2], func=AF.Sqrt)
                # epilogue: S copy, norm, exp(-norm)
                scalar.wait_ge(sem_pe, 1)
                scalar.activation(out=s[:], in_=ps[:], func=AF.Copy).then_inc(
                    sem_s1, 1
                )
                scalar.wait_ge(sem_v2, 1)
                scalar.activation(out=normt[:], in_=ss[:], func=AF.Sqrt).then_inc(
                    sem_act1, 1
                )
                scalar.wait_ge(sem_act1, 1)
                scalar.activation(
                    out=et[:], in_=normt[:], func=AF.Exp, scale=-1.0
                ).then_inc(sem_act2, 1)

    return nc


_CACHE = {}


def _get_nc():
    if "nc" not in _CACHE:
        _CACHE["nc"] = build_raw()
    return _CACHE["nc"]


def prep_inputs(primary_caps, W, B):
    """Host-side layout prep + sharding (no arithmetic).

    Contraction row order: chunk c holds n in [c*16, (c+1)*16); within a
    chunk, partition p = j*16 + n_local.  Core c owns digit caps
    d in {2c, 2c+1} (zeros for the 6 pad slots on cores 5-7).
    """
    U = np.asarray(primary_caps, dtype=np.float32)
    Wf = np.asarray(W, dtype=np.float32)
    Bf = np.asarray(B, dtype=np.float32).reshape(D, N)

    # U^T replicated: [p, (c b)]
    Unj = np.transpose(U, (1, 2, 0))  # n j b
    Ut = np.ascontiguousarray(
        Unj.reshape(NCHUNK, 16, DP, BFULL)
        .transpose(0, 2, 1, 3)
        .reshape(NCHUNK, P, BFULL)
        .transpose(1, 0, 2)
        .reshape(P, NCHUNK * BFULL)
    )

    # per-core W slice [p, (c, t, i)] and B slice [p, (c, t)]
    Wnj = np.transpose(Wf, (1, 3, 0, 2))  # n j d i
    Wc = (
        Wnj.reshape(NCHUNK, 16, DP, D, DD)
        .transpose(0, 2, 1, 3, 4)          # c j n_l d i
        .reshape(NCHUNK, P, D, DD)
        .transpose(1, 0, 2, 3)             # p c d i
    )
    Bn = Bf.reshape(D, NCHUNK, 16)         # d c n_l

    in_maps = []
    for core in range(NCORES):
        wt = np.zeros((P, NCHUNK, DC, DD), dtype=np.float32)
        bpt = np.zeros((16, NCHUNK, DC), dtype=np.float32)
        for t in range(DC):
            d = 2 * core + t
            if d < D:
                wt[:, :, t, :] = Wc[:, :, d, :]
                bpt[:, :, t] = Bn[d].T      # [n_l, c] -> ...
        bpm = np.ascontiguousarray(
            np.broadcast_to(
                bpt.reshape(1, 16, NCHUNK * DC), (DP, 16, NCHUNK * DC)
            ).reshape(P, NCHUNK * DC)
        )
        in_maps.append(
            {
                "u_t": Ut,
                "w_t": np.ascontiguousarray(wt.reshape(P, NCHUNK * DIC)),
                "bp": bpm,
            }
        )
    return in_maps


def kernel(primary_caps, W, B):
    nc = _get_nc()
    in_maps = prep_inputs(primary_caps, W, B)
    res = run_bass_kernel_spmd(nc, in_maps, core_ids=list(range(NCORES)))
    full = np.empty((BFULL, D, DD), dtype=np.float32)
    for core in range(NCORES):
        o = res.results[core]["out"].reshape(BFULL, DC, DD)
        for t in range(DC):
            d = 2 * core + t
            if d < D:
                full[:, d, :] = o[:, t, :]
    return full
